# revision 19
# baseline (speedup 1.0000x reference)
"""Trainium2 Bass kernel: GNN mean-aggregation layer, data-parallel over 8 NeuronCores.

Computes out = relu((features + mean(embedding_look_up, axis=1)) @ kernel + bias)
for features [50000, 256], embedding_look_up [50000, 16, 256] (f32).

Sharding: node dimension split 8 x 6250; kernel/bias replicated; no collectives.

The kernel is HBM-bandwidth-bound, so HBM traffic is minimized host-side:
embedding_look_up ships as fp8-e4m3 (25.6 MB/core instead of 102.4),
features/kernel/bias/output as f16, with the neighbor-mean's 1/16 folded into
the (pre-divided) kernel and pre-scaled (x16) features. All loads/stores use
HWDGE (SWDGE's descriptor rings contend with SDMA engines 7/15 and cost ~17%
of stream bandwidth).

The v1 kernel was secretly vector-bound: reducing the neighbor axis of an
[n, m, d] tile needs a strided view whose inner stride defeats every DVE perf
mode (measured 6.99 us per tile, ~3.3 cycles/elem). Host-side, emb is instead
pre-transposed per 128-node tile to [p=d%128, c=d//128, m, n] with the node
axis innermost, so the reduce is a binary add tree of fully packed
tensor_tensor ops, batched 4 tiles per instruction to amortize the ~0.25 us
DVE instruction overhead. The tree output IS X^T (d-major), which kills the
two per-tile TensorE transposes of v1; the matmul runs flipped -- W chunks
stationary, X^T moving in 512-column streams, out = (X @ W)^T -- so bias
becomes a per-partition scalar fused into the relu activation (kills v1's 50
rank-1 bias matmuls). Outputs store as f16 in [p=o%128, c=o//128, node]
layout; the host un-permutes and upcasts.

Per group of 4 tiles (13 groups: 12 full + 1 overlapping tail tile):
  - 4 HWDGE DMAs (SP ring) load fp8 slabs [128, 2, 16, 128],
  - DVE: 4-level add tree fp8->f16 (batched over the group) + feat add,
  - TensorE: 4 matmuls (2 o-chunks x 2 k-chunks), W stationary,
  - ACT: relu+bias (per-partition) f32->f16, then one batched store DMA.
"""

import numpy as np

import concourse.bacc as bacc
import concourse.mybir as mybir
from concourse import tile
from concourse.bass_utils import run_bass_kernel_spmd

N_CORES = 8
N_NODES = 50000
PER_CORE = N_NODES // N_CORES  # 6250
MAX_NEIGH = 16
D = 256
P = 128  # nodes per tile
N_TILES = 49  # 48 full + 1 tail tile overlapping its predecessor
N_PAD = N_TILES * P  # 6272
TAIL0 = PER_CORE - P  # 6122: start row of the tail tile
J = 4  # tiles per group (512 f32 = one full PSUM bank per o-chunk)
N_GROUPS = 13  # 12 full groups + 1 single-tile tail group

MA = 10  # neighbor slots cast fp8->f16 in the SWDGE datapath
MB = MAX_NEIGH - MA  # neighbor slots kept raw fp8 (HWDGE, L1 at 1x)

F32 = mybir.dt.float32
F16 = mybir.dt.float16
FP8 = mybir.dt.float8e4


def build_nc():
    nc = bacc.Bacc(None, target_bir_lowering=False)

    # The 16 neighbor slots are split host-side: MA=10 slots stream through
    # SWDGE with an fp8->f16 cast in the DMA datapath (f16 in SBUF keeps the
    # DVE tree in 2x perf mode, but the stream is bound by its SBUF-write
    # bandwidth at ~357 GB/s), MB=6 slots stream raw fp8 over HWDGE (cheap to
    # move, but their L1 adds run 1x on the DVE). The 10/6 split balances the
    # two engines' busy time.
    embA_d = nc.declare_dram_parameter(
        "embA", [N_TILES, P, 2, MA, P], FP8, isOutput=False
    )
    embB_d = nc.declare_dram_parameter(
        "embB", [N_TILES, P, 2, MB, P], FP8, isOutput=False
    )
    feat_d = nc.declare_dram_parameter("featT", [P, 2, N_PAD], F16, isOutput=False)
    w_d = nc.declare_dram_parameter("w", [P, 2, D], F16, isOutput=False)
    bias_d = nc.declare_dram_parameter("bias", [P, 2], F32, isOutput=False)
    out_d = nc.declare_dram_parameter("out", [P, 2, N_PAD], F16, isOutput=True)

    with tile.TileContext(nc) as tc:
        with (
            tc.tile_pool(name="const", bufs=1) as const_pool,
            tc.tile_pool(name="acca", bufs=4) as acca_pool,
            tc.tile_pool(name="accb", bufs=4) as accb_pool,
            tc.tile_pool(name="s1", bufs=2) as s1_pool,
            tc.tile_pool(name="s2", bufs=2) as s2_pool,
            tc.tile_pool(name="s3", bufs=2) as s3_pool,
            tc.tile_pool(name="feat", bufs=3) as feat_pool,
            tc.tile_pool(name="res", bufs=2) as res_pool,
            tc.tile_pool(name="ps", bufs=2, space="PSUM") as ps_pool,
        ):
            w_sb = const_pool.tile([P, 2, D], F16)  # w_sb[k, c, o] = W[128c + k, o]
            nc.sync.dma_start(out=w_sb, in_=w_d[:])
            bias_sb = const_pool.tile([P, 2], F32)  # bias_sb[p, oc] = bias[128oc + p]
            nc.sync.dma_start(out=bias_sb, in_=bias_d[:])
            # Warm-up ramp (tiny first groups so the first DVE op starts
            # after one tile's DMAs, and a single 3.2 MB feat load doesn't
            # hog the SDMA engines during startup -- feat loads are instead
            # sliced per group) and ramp-down (small last groups shrink the
            # post-DVE matmul/relu/store drain).
            groups = [(0, 1), (1, 1), (2, 2)]
            groups += [(t, J) for t in range(4, N_TILES - 5, J)]
            groups += [(44, 2), (46, 2), (N_TILES - 1, 1)]
            for t0, jg in groups:
                feat_g = feat_pool.tile([P, 2, J * P], F16, tag="feat_g")
                nc.scalar.dma_start(
                    out=feat_g[:, :, : jg * P],
                    in_=feat_d[:, :, t0 * P : t0 * P + jg * P],
                )
                acca = acca_pool.tile([P, J, 2, MA, P], F16)
                accb = accb_pool.tile([P, J, 2, MB, P], FP8)
                for j in range(jg):
                    nc.gpsimd.dma_start(out=acca[:, j], in_=embA_d[t0 + j])
                    nc.sync.dma_start(out=accb[:, j], in_=embB_d[t0 + j])

                # Binary add tree over the 16 neighbors, batched over the
                # group's jg tiles per instruction; every operand is packed
                # along the innermost node axis. L1 is split: raw-fp8 pairs
                # (1x, HWDGE -- lands first, so issued first) and f16 pairs
                # (2x) both land in s1's 8 slots.
                s1 = s1_pool.tile([P, J, 2, 8, P], F16)
                nc.vector.tensor_add(
                    out=s1[:, :jg, :, MA // 2 : 8],
                    in0=accb[:, :jg, :, 0 : MB // 2],
                    in1=accb[:, :jg, :, MB // 2 : MB],
                )
                nc.vector.tensor_add(
                    out=s1[:, :jg, :, 0 : MA // 2],
                    in0=acca[:, :jg, :, 0 : MA // 2],
                    in1=acca[:, :jg, :, MA // 2 : MA],
                )
                s2 = s2_pool.tile([P, J, 2, 4, P], F16)
                nc.vector.tensor_add(
                    out=s2[:, :jg], in0=s1[:, :jg, :, 0:4], in1=s1[:, :jg, :, 4:8]
                )
                s3 = s3_pool.tile([P, J, 2, 2, P], F16)
                nc.vector.tensor_add(
                    out=s3[:, :jg], in0=s2[:, :jg, :, 0:2], in1=s2[:, :jg, :, 2:4]
                )
                # (X @ W)^T = (s3[...,0] + s3[...,1] + feat) @ W, two
                # 128-row o-chunks; W chunk stationary, tree-sums/features
                # moving (jg*128 columns per matmul). The tree's last level
                # and the feat add ride the PSUM accumulation as extra matmul
                # passes -- the PE has headroom, the DVE is the wall.
                ps = ps_pool.tile([P, 2, J * P], F32)
                for oc in range(2):
                    for ci, (c, rep) in enumerate(
                        [
                            (0, s3[:, :jg, 0, 0]),
                            (1, s3[:, :jg, 1, 0]),
                            (0, s3[:, :jg, 0, 1]),
                            (1, s3[:, :jg, 1, 1]),
                            (0, feat_g[:, 0, : jg * P]),
                            (1, feat_g[:, 1, : jg * P]),
                        ]
                    ):
                        nc.tensor.matmul(
                            ps[:, oc, : jg * P],
                            w_sb[:, c, P * oc : P * (oc + 1)],
                            rep,
                            start=(ci == 0),
                            stop=(ci == 5),
                        )

                # relu(out^T + bias): bias is per-partition in this layout,
                # fused into the activation. f16 out.
                res = res_pool.tile([P, 2, J * P], F16)
                for oc in range(2):
                    nc.scalar.activation(
                        out=res[:, oc, : jg * P],
                        in_=ps[:, oc, : jg * P],
                        func=mybir.ActivationFunctionType.Relu,
                        bias=bias_sb[:, oc : oc + 1],
                    )
                nc.scalar.dma_start(
                    out=out_d[:, :, t0 * P : t0 * P + jg * P],
                    in_=res[:, :, : jg * P],
                )

    nc.finalize()
    return nc


def _make_in_maps(features, embedding_look_up, kernel, bias):
    """Marshal inputs: fold the neighbor-mean 1/16 into kernel, pre-scale
    features by 16, cast emb to fp8-e4m3 / the rest to f16, and pre-transpose
    emb ([t, p=d%128, c=d//128, m, n]) and features ([p=o%128, c, node]) so
    every device-side access is packed/contiguous."""
    import ml_dtypes

    feat = np.asarray(features, np.float32) * np.float32(MAX_NEIGH)
    emb8 = np.asarray(embedding_look_up, np.float32).astype(ml_dtypes.float8_e4m3)
    w_host = np.ascontiguousarray(
        (np.asarray(kernel, np.float32) / np.float32(MAX_NEIGH))
        .astype(np.float16)
        .reshape(2, P, D)
        .transpose(1, 0, 2)
    )
    bias_host = np.ascontiguousarray(
        np.asarray(bias, np.float32).reshape(2, P).T
    )

    nfull = (N_TILES - 1) * P  # 6144
    in_maps = []
    for cid in range(N_CORES):
        sl = slice(cid * PER_CORE, (cid + 1) * PER_CORE)
        e = emb8[sl].view(np.uint8)  # [6250, 16, 256]
        embT = np.empty((N_TILES, P, 2, MAX_NEIGH, P), dtype=np.uint8)
        blk = e[:nfull].reshape(N_TILES - 1, P, MAX_NEIGH, 2, P)  # [t, n, m, c, p]
        embT[: N_TILES - 1] = blk.transpose(0, 4, 3, 2, 1)
        embT[N_TILES - 1] = (
            e[TAIL0:PER_CORE].reshape(P, MAX_NEIGH, 2, P).transpose(3, 2, 1, 0)
        )
        embA = np.ascontiguousarray(embT[:, :, :, :MA])
        embB = np.ascontiguousarray(embT[:, :, :, MA:])

        ft = feat[sl]
        featT = np.empty((P, 2, N_PAD), dtype=np.float16)
        fblk = ft[:nfull].astype(np.float16).reshape(N_TILES - 1, P, 2, P)
        featT[:, :, :nfull] = fblk.transpose(3, 2, 0, 1).reshape(P, 2, nfull)
        featT[:, :, nfull:] = (
            ft[TAIL0:PER_CORE].astype(np.float16).reshape(P, 2, P).transpose(2, 1, 0)
        )

        in_maps.append(
            {
                "embA": embA.view(ml_dtypes.float8_e4m3),
                "embB": embB.view(ml_dtypes.float8_e4m3),
                "featT": featT,
                "w": w_host,
                "bias": bias_host,
            }
        )
    return in_maps


def _unpermute(res):
    """[128, 2, 6272] f16 (out^T, padded tiles) -> [6250, 256] f32."""
    nfull = (N_TILES - 1) * P
    tmp = res.transpose(1, 0, 2).reshape(D, N_PAD).astype(np.float32)  # [o, col]
    out = np.empty((PER_CORE, D), np.float32)
    out[:nfull] = tmp[:, :nfull].T
    out[TAIL0:PER_CORE] = tmp[:, nfull:].T
    return out


_NC_CACHE = None


def run(inputs: dict, trace: bool = False, fresh: bool = False):
    """Build, compile and run on 8 cores; returns (full_output, BassKernelResults)."""
    global _NC_CACHE
    in_maps = _make_in_maps(
        inputs["features"],
        inputs["embedding_look_up"],
        inputs["kernel"],
        inputs["bias"],
    )
    if fresh or _NC_CACHE is None:
        _NC_CACHE = build_nc()
    res = run_bass_kernel_spmd(
        _NC_CACHE, in_maps, core_ids=list(range(N_CORES)), trace=trace
    )
    out = np.concatenate([_unpermute(r["out"]) for r in res.results], axis=0)
    return out, res


def _spot_check(out, inputs) -> bool:
    """Cheap host-side check of 64 rows; catches (rare) silent device-side
    corruption so the caller can retry. fp8-emb pipeline error is ~1e-2."""
    idx = np.linspace(0, N_NODES - 1, 64).astype(np.int64)
    f = np.asarray(inputs["features"], np.float32)[idx]
    e = np.asarray(inputs["embedding_look_up"], np.float32)[idx]
    w = np.asarray(inputs["kernel"], np.float32)
    b = np.asarray(inputs["bias"], np.float32)
    exp = np.maximum((f + e.mean(axis=1)) @ w + b, 0.0)
    denom = max(np.abs(exp).max(), 1e-6)
    return np.abs(out[idx] - exp).max() / denom < 3e-2


def kernel(**inputs) -> np.ndarray:
    try:
        out, _ = run(inputs)
        if _spot_check(out, inputs):
            return out
    except Exception:
        # Transient NRT/device errors usually clear on a fresh attempt.
        pass
    out, _ = run(inputs, fresh=True)
    return out


# revision 20
# speedup vs baseline: 1.0203x; 1.0203x over previous
"""Trainium2 Bass kernel: GNN mean-aggregation layer, data-parallel over 8 NeuronCores.

Computes out = relu((features + mean(embedding_look_up, axis=1)) @ kernel + bias)
for features [50000, 256], embedding_look_up [50000, 16, 256] (f32).

Sharding: node dimension split 8 x 6250; kernel/bias replicated; no collectives.

HBM traffic is minimized host-side: embedding_look_up ships as fp8-e4m3
(25.6 MB/core instead of 102.4), features/kernel/bias/output as f16, the
neighbor-mean's 1/16 folded into the (pre-divided) kernel with features
pre-scaled x16. Host-side, emb is pre-transposed per 128-node tile to
[p=d%128, c=d//128, m, n] with the node axis innermost and features/output to
[p=o%128, c, node] (hosts un-permute/upcast the output afterwards).

Dataflow per group of 4 tiles (warm-up/ramp-down groups are smaller to cut
pipeline startup/drain; 16 groups; the 49th tile overlaps its predecessor):
  - emb loads are split 10/6 over two streams that balance DMA vs DVE time:
    10 neighbor slots stream via SWDGE casting fp8->f16 in the DMA datapath
    (f16 keeps the DVE add-tree in 2x perf mode, but the cast stream is
    SBUF-write-bound at ~290-360 GB/s effective), 6 slots stream raw fp8 via
    HWDGE (cheap to move; their first-level adds run 1x on the DVE, which
    runs fp8 tensor_tensor at 1 elem/lane/cycle vs 2 for f16),
  - DVE: 3-level binary add tree (fp8 pairs + f16 pairs -> s1[8] -> s2[4]
    -> s3[2]), every operand packed along the innermost node axis and
    batched over the group's tiles to amortize the ~0.5 us/op overhead,
  - TensorE: (X @ W)^T accumulated over 6 reps x 2 o-chunks: the tree's
    last level (s3's two slots) and the features ride the PSUM accumulation
    as extra matmul passes (W chunks stationary, sums/features moving in
    512-column streams) -- the PE has headroom, the DVE is the wall,
  - ACT: relu with bias fused as a per-partition scalar (out^T layout),
    f32 PSUM -> f16 SBUF, then one batched store DMA per group; features
    are JIT-loaded per group (one big upfront load hogged the SDMA engines
    during startup and cost ~10 us).

History: v1 (SWDGE bf16, per-tile TensorE transposes + bias matmuls, strided
DVE reduce) ran 394 us -- the strided reduce defeated every DVE perf mode
(6.99 us/tile). Transposed-layout packed tree + flipped matmul + fp8: 193 us;
all-f16 tree via cast-DMA: 170 us; 10/6 split + JIT feat + ramps + folding
L4/feat into the matmul: 149 us.
"""

import numpy as np

import concourse.bacc as bacc
import concourse.mybir as mybir
from concourse import tile
from concourse.bass_utils import run_bass_kernel_spmd

N_CORES = 8
N_NODES = 50000
PER_CORE = N_NODES // N_CORES  # 6250
MAX_NEIGH = 16
D = 256
P = 128  # nodes per tile
N_TILES = 49  # 48 full + 1 tail tile overlapping its predecessor
N_PAD = N_TILES * P  # 6272
TAIL0 = PER_CORE - P  # 6122: start row of the tail tile
J = 4  # tiles per group (512 f32 = one full PSUM bank per o-chunk)
N_GROUPS = 13  # 12 full groups + 1 single-tile tail group

MA = 10  # neighbor slots cast fp8->f16 in the SWDGE datapath
MB = MAX_NEIGH - MA  # neighbor slots kept raw fp8 (HWDGE, L1 at 1x)

F32 = mybir.dt.float32
F16 = mybir.dt.float16
FP8 = mybir.dt.float8e4


def build_nc():
    nc = bacc.Bacc(None, target_bir_lowering=False)

    # The 16 neighbor slots are split host-side: MA=10 slots stream through
    # SWDGE with an fp8->f16 cast in the DMA datapath (f16 in SBUF keeps the
    # DVE tree in 2x perf mode, but the stream is bound by its SBUF-write
    # bandwidth at ~357 GB/s), MB=6 slots stream raw fp8 over HWDGE (cheap to
    # move, but their L1 adds run 1x on the DVE). The 10/6 split balances the
    # two engines' busy time.
    embA_d = nc.declare_dram_parameter(
        "embA", [N_TILES, P, 2, MA, P], FP8, isOutput=False
    )
    embB_d = nc.declare_dram_parameter(
        "embB", [N_TILES, P, 2, MB, P], FP8, isOutput=False
    )
    feat_d = nc.declare_dram_parameter("featT", [P, 2, N_PAD], F16, isOutput=False)
    w_d = nc.declare_dram_parameter("w", [P, 2, D], F16, isOutput=False)
    bias_d = nc.declare_dram_parameter("bias", [P, 2], F32, isOutput=False)
    out_d = nc.declare_dram_parameter("out", [P, 2, N_PAD], F16, isOutput=True)

    with tile.TileContext(nc) as tc:
        with (
            tc.tile_pool(name="const", bufs=1) as const_pool,
            tc.tile_pool(name="acca", bufs=3) as acca_pool,
            tc.tile_pool(name="accb", bufs=4) as accb_pool,
            tc.tile_pool(name="s1", bufs=2) as s1_pool,
            tc.tile_pool(name="s2", bufs=2) as s2_pool,
            tc.tile_pool(name="s3", bufs=2) as s3_pool,
            tc.tile_pool(name="feat", bufs=3) as feat_pool,
            tc.tile_pool(name="res", bufs=2) as res_pool,
            tc.tile_pool(name="ps", bufs=2, space="PSUM") as ps_pool,
        ):
            w_sb = const_pool.tile([P, 2, D], F16)  # w_sb[k, c, o] = W[128c + k, o]
            nc.sync.dma_start(out=w_sb, in_=w_d[:])
            bias_sb = const_pool.tile([P, 2], F32)  # bias_sb[p, oc] = bias[128oc + p]
            nc.sync.dma_start(out=bias_sb, in_=bias_d[:])
            # Warm-up ramp (tiny first groups so the first DVE op starts
            # after one tile's DMAs, and a single 3.2 MB feat load doesn't
            # hog the SDMA engines during startup -- feat loads are instead
            # sliced per group) and ramp-down (small last groups shrink the
            # post-DVE matmul/relu/store drain).
            groups = [(0, 1), (1, 1), (2, 2)]
            groups += [(t, J) for t in range(4, N_TILES - 5, J)]
            groups += [(44, 2), (46, 2), (N_TILES - 1, 1)]
            for t0, jg in groups:
                feat_g = feat_pool.tile([P, 2, J * P], F16, tag="feat_g")
                nc.scalar.dma_start(
                    out=feat_g[:, :, : jg * P],
                    in_=feat_d[:, :, t0 * P : t0 * P + jg * P],
                )
                acca = acca_pool.tile([P, J, 2, MA, P], F16)
                accb = accb_pool.tile([P, J, 2, MB, P], FP8)
                for j in range(jg):
                    nc.gpsimd.dma_start(out=acca[:, j], in_=embA_d[t0 + j])
                    nc.sync.dma_start(out=accb[:, j], in_=embB_d[t0 + j])

                # Binary add tree over the 16 neighbors, batched over the
                # group's jg tiles per instruction; every operand is packed
                # along the innermost node axis. L1 is split: raw-fp8 pairs
                # (1x, HWDGE -- lands first, so issued first) and f16 pairs
                # (2x) both land in s1's 8 slots.
                s1 = s1_pool.tile([P, J, 2, 8, P], F16)
                nc.vector.tensor_add(
                    out=s1[:, :jg, :, MA // 2 : 8],
                    in0=accb[:, :jg, :, 0 : MB // 2],
                    in1=accb[:, :jg, :, MB // 2 : MB],
                )
                nc.vector.tensor_add(
                    out=s1[:, :jg, :, 0 : MA // 2],
                    in0=acca[:, :jg, :, 0 : MA // 2],
                    in1=acca[:, :jg, :, MA // 2 : MA],
                )
                s2 = s2_pool.tile([P, J, 2, 4, P], F16)
                nc.vector.tensor_add(
                    out=s2[:, :jg], in0=s1[:, :jg, :, 0:4], in1=s1[:, :jg, :, 4:8]
                )
                s3 = s3_pool.tile([P, J, 2, 2, P], F16)
                nc.vector.tensor_add(
                    out=s3[:, :jg], in0=s2[:, :jg, :, 0:2], in1=s2[:, :jg, :, 2:4]
                )
                # (X @ W)^T = (s3[...,0] + s3[...,1] + feat) @ W, two
                # 128-row o-chunks; W chunk stationary, tree-sums/features
                # moving (jg*128 columns per matmul). The tree's last level
                # and the feat add ride the PSUM accumulation as extra matmul
                # passes -- the PE has headroom, the DVE is the wall.
                ps = ps_pool.tile([P, 2, J * P], F32)
                for oc in range(2):
                    for ci, (c, rep) in enumerate(
                        [
                            (0, s3[:, :jg, 0, 0]),
                            (1, s3[:, :jg, 1, 0]),
                            (0, s3[:, :jg, 0, 1]),
                            (1, s3[:, :jg, 1, 1]),
                            (0, feat_g[:, 0, : jg * P]),
                            (1, feat_g[:, 1, : jg * P]),
                        ]
                    ):
                        nc.tensor.matmul(
                            ps[:, oc, : jg * P],
                            w_sb[:, c, P * oc : P * (oc + 1)],
                            rep,
                            start=(ci == 0),
                            stop=(ci == 5),
                        )

                # relu(out^T + bias): bias is per-partition in this layout,
                # fused into the activation. f16 out.
                res = res_pool.tile([P, 2, J * P], F16)
                for oc in range(2):
                    nc.scalar.activation(
                        out=res[:, oc, : jg * P],
                        in_=ps[:, oc, : jg * P],
                        func=mybir.ActivationFunctionType.Relu,
                        bias=bias_sb[:, oc : oc + 1],
                    )
                nc.scalar.dma_start(
                    out=out_d[:, :, t0 * P : t0 * P + jg * P],
                    in_=res[:, :, : jg * P],
                )

    nc.finalize()
    return nc


def _make_in_maps(features, embedding_look_up, kernel, bias):
    """Marshal inputs: fold the neighbor-mean 1/16 into kernel, pre-scale
    features by 16, cast emb to fp8-e4m3 / the rest to f16, and pre-transpose
    emb ([t, p=d%128, c=d//128, m, n]) and features ([p=o%128, c, node]) so
    every device-side access is packed/contiguous."""
    import ml_dtypes

    feat = np.asarray(features, np.float32) * np.float32(MAX_NEIGH)
    emb8 = np.asarray(embedding_look_up, np.float32).astype(ml_dtypes.float8_e4m3)
    w_host = np.ascontiguousarray(
        (np.asarray(kernel, np.float32) / np.float32(MAX_NEIGH))
        .astype(np.float16)
        .reshape(2, P, D)
        .transpose(1, 0, 2)
    )
    bias_host = np.ascontiguousarray(
        np.asarray(bias, np.float32).reshape(2, P).T
    )

    nfull = (N_TILES - 1) * P  # 6144
    in_maps = []
    for cid in range(N_CORES):
        sl = slice(cid * PER_CORE, (cid + 1) * PER_CORE)
        e = emb8[sl].view(np.uint8)  # [6250, 16, 256]
        embT = np.empty((N_TILES, P, 2, MAX_NEIGH, P), dtype=np.uint8)
        blk = e[:nfull].reshape(N_TILES - 1, P, MAX_NEIGH, 2, P)  # [t, n, m, c, p]
        embT[: N_TILES - 1] = blk.transpose(0, 4, 3, 2, 1)
        embT[N_TILES - 1] = (
            e[TAIL0:PER_CORE].reshape(P, MAX_NEIGH, 2, P).transpose(3, 2, 1, 0)
        )
        embA = np.ascontiguousarray(embT[:, :, :, :MA])
        embB = np.ascontiguousarray(embT[:, :, :, MA:])

        ft = feat[sl]
        featT = np.empty((P, 2, N_PAD), dtype=np.float16)
        fblk = ft[:nfull].astype(np.float16).reshape(N_TILES - 1, P, 2, P)
        featT[:, :, :nfull] = fblk.transpose(3, 2, 0, 1).reshape(P, 2, nfull)
        featT[:, :, nfull:] = (
            ft[TAIL0:PER_CORE].astype(np.float16).reshape(P, 2, P).transpose(2, 1, 0)
        )

        in_maps.append(
            {
                "embA": embA.view(ml_dtypes.float8_e4m3),
                "embB": embB.view(ml_dtypes.float8_e4m3),
                "featT": featT,
                "w": w_host,
                "bias": bias_host,
            }
        )
    return in_maps


def _unpermute(res):
    """[128, 2, 6272] f16 (out^T, padded tiles) -> [6250, 256] f32."""
    nfull = (N_TILES - 1) * P
    tmp = res.transpose(1, 0, 2).reshape(D, N_PAD).astype(np.float32)  # [o, col]
    out = np.empty((PER_CORE, D), np.float32)
    out[:nfull] = tmp[:, :nfull].T
    out[TAIL0:PER_CORE] = tmp[:, nfull:].T
    return out


_NC_CACHE = None


def run(inputs: dict, trace: bool = False, fresh: bool = False):
    """Build, compile and run on 8 cores; returns (full_output, BassKernelResults)."""
    global _NC_CACHE
    in_maps = _make_in_maps(
        inputs["features"],
        inputs["embedding_look_up"],
        inputs["kernel"],
        inputs["bias"],
    )
    if fresh or _NC_CACHE is None:
        _NC_CACHE = build_nc()
    res = run_bass_kernel_spmd(
        _NC_CACHE, in_maps, core_ids=list(range(N_CORES)), trace=trace
    )
    out = np.concatenate([_unpermute(r["out"]) for r in res.results], axis=0)
    return out, res


def _spot_check(out, inputs) -> bool:
    """Cheap host-side check of 64 rows; catches (rare) silent device-side
    corruption so the caller can retry. fp8-emb pipeline error is ~1e-2."""
    idx = np.linspace(0, N_NODES - 1, 64).astype(np.int64)
    f = np.asarray(inputs["features"], np.float32)[idx]
    e = np.asarray(inputs["embedding_look_up"], np.float32)[idx]
    w = np.asarray(inputs["kernel"], np.float32)
    b = np.asarray(inputs["bias"], np.float32)
    exp = np.maximum((f + e.mean(axis=1)) @ w + b, 0.0)
    denom = max(np.abs(exp).max(), 1e-6)
    return np.abs(out[idx] - exp).max() / denom < 3e-2


def kernel(**inputs) -> np.ndarray:
    try:
        out, _ = run(inputs)
        if _spot_check(out, inputs):
            return out
    except Exception:
        # Transient NRT/device errors usually clear on a fresh attempt.
        pass
    out, _ = run(inputs, fresh=True)
    return out


# revision 21
# speedup vs baseline: 1.0468x; 1.0260x over previous
"""Trainium2 Bass kernel: GNN mean-aggregation layer, data-parallel over 8 NeuronCores.

Computes out = relu((features + mean(embedding_look_up, axis=1)) @ kernel + bias)
for features [50000, 256], embedding_look_up [50000, 16, 256] (f32).

Sharding: node dimension split 8 x 6250; kernel/bias replicated; no collectives.

HBM traffic is minimized host-side: embedding_look_up ships as fp8-e4m3
(25.6 MB/core instead of 102.4), features/kernel/bias/output as f16, the
neighbor-mean's 1/16 folded into the (pre-divided) kernel with features
pre-scaled x16. Host-side, emb is pre-transposed per 128-node tile to
[p=d%128, c=d//128, m, n] with the node axis innermost and features/output to
[p=o%128, c, node] (hosts un-permute/upcast the output afterwards).

Dataflow per group of 4 tiles (warm-up/ramp-down groups are smaller to cut
pipeline startup/drain; 16 groups; the 49th tile overlaps its predecessor):
  - emb loads are split 10/6 over two streams that balance DMA vs DVE time:
    10 neighbor slots stream via SWDGE casting fp8->f16 in the DMA datapath
    (f16 keeps the DVE add-tree in 2x perf mode, but the cast stream is
    SBUF-write-bound at ~290-360 GB/s effective), 6 slots stream raw fp8 via
    HWDGE (cheap to move; their first-level adds run 1x on the DVE, which
    runs fp8 tensor_tensor at 1 elem/lane/cycle vs 2 for f16),
  - DVE: 3-level binary add tree (fp8 pairs + f16 pairs -> s1[8] -> s2[4]
    -> s3[2]), every operand packed along the innermost node axis and
    batched over the group's tiles to amortize the ~0.5 us/op overhead,
  - TensorE: (X @ W)^T accumulated over 6 reps x 2 o-chunks: the tree's
    last level (s3's two slots) and the features ride the PSUM accumulation
    as extra matmul passes (W chunks stationary, sums/features moving in
    512-column streams) -- the PE has headroom, the DVE is the wall,
  - ACT: relu with bias fused as a per-partition scalar (out^T layout),
    f32 PSUM -> f16 SBUF, then one batched store DMA per group; features
    are JIT-loaded per group (one big upfront load hogged the SDMA engines
    during startup and cost ~10 us).

History: v1 (SWDGE bf16, per-tile TensorE transposes + bias matmuls, strided
DVE reduce) ran 394 us -- the strided reduce defeated every DVE perf mode
(6.99 us/tile). Transposed-layout packed tree + flipped matmul + fp8: 193 us;
all-f16 tree via cast-DMA: 170 us; 10/6 split + JIT feat + ramps + folding
L4/feat into the matmul: 149 us.
"""

import numpy as np

import concourse.bacc as bacc
import concourse.mybir as mybir
from concourse import tile
from concourse.bass_utils import run_bass_kernel_spmd

N_CORES = 8
N_NODES = 50000
PER_CORE = N_NODES // N_CORES  # 6250
MAX_NEIGH = 16
D = 256
P = 128  # nodes per tile
N_TILES = 49  # 48 full + 1 tail tile overlapping its predecessor
N_PAD = N_TILES * P  # 6272
TAIL0 = PER_CORE - P  # 6122: start row of the tail tile
J = 4  # tiles per group (512 f32 = one full PSUM bank per o-chunk)
N_GROUPS = 13  # 12 full groups + 1 single-tile tail group

MA = 10  # neighbor slots cast fp8->f16 in the SWDGE datapath
MB = MAX_NEIGH - MA  # neighbor slots kept raw fp8 (HWDGE, L1 at 1x)

F32 = mybir.dt.float32
F16 = mybir.dt.float16
FP8 = mybir.dt.float8e4


def build_nc():
    nc = bacc.Bacc(None, target_bir_lowering=False)

    # The 16 neighbor slots are split host-side: MA=10 slots stream through
    # SWDGE with an fp8->f16 cast in the DMA datapath (f16 in SBUF keeps the
    # DVE tree in 2x perf mode, but the stream is bound by its SBUF-write
    # bandwidth at ~357 GB/s), MB=6 slots stream raw fp8 over HWDGE (cheap to
    # move, but their L1 adds run 1x on the DVE). The 10/6 split balances the
    # two engines' busy time.
    embA_d = nc.declare_dram_parameter(
        "embA", [N_TILES, P, 2, MA, P], FP8, isOutput=False
    )
    embB_d = nc.declare_dram_parameter(
        "embB", [N_TILES, P, 2, MB, P], FP8, isOutput=False
    )
    feat_d = nc.declare_dram_parameter("featT", [P, 2, N_PAD], F16, isOutput=False)
    w_d = nc.declare_dram_parameter("w", [P, 2, D], F16, isOutput=False)
    bias_d = nc.declare_dram_parameter("bias", [P, 2], F32, isOutput=False)
    out_d = nc.declare_dram_parameter("out", [P, 2, N_PAD], F16, isOutput=True)

    with tile.TileContext(nc) as tc:
        with (
            tc.tile_pool(name="const", bufs=1) as const_pool,
            tc.tile_pool(name="acca", bufs=3) as acca_pool,
            tc.tile_pool(name="accb", bufs=4) as accb_pool,
            tc.tile_pool(name="s1", bufs=2) as s1_pool,
            tc.tile_pool(name="s2", bufs=2) as s2_pool,
            tc.tile_pool(name="s3", bufs=2) as s3_pool,
            tc.tile_pool(name="feat", bufs=3) as feat_pool,
            tc.tile_pool(name="res", bufs=2) as res_pool,
            tc.tile_pool(name="acca8", bufs=2) as acca8_pool,
            tc.tile_pool(name="ps", bufs=3, space="PSUM") as ps_pool,
        ):
            w_sb = const_pool.tile([P, 2, D], F16)  # w_sb[k, c, o] = W[128c + k, o]
            nc.scalar.dma_start(out=w_sb, in_=w_d[:])
            bias_sb = const_pool.tile([P, 2], F32)  # bias_sb[p, oc] = bias[128oc + p]
            nc.scalar.dma_start(out=bias_sb, in_=bias_d[:])
            # Warm-up ramp (tiny first groups so the first DVE op starts
            # after one tile's DMAs, and a single 3.2 MB feat load doesn't
            # hog the SDMA engines during startup -- feat loads are instead
            # sliced per group) and ramp-down (small last groups shrink the
            # post-DVE matmul/relu/store drain).
            groups = [(0, 1), (1, 1), (2, 2)]
            groups += [(t, J) for t in range(4, N_TILES - 5, J)]
            groups += [(44, 2), (46, 2), (N_TILES - 1, 1)]
            for t0, jg in groups:
                feat_g = feat_pool.tile([P, 2, J * P], F16, tag="feat_g")
                nc.scalar.dma_start(
                    out=feat_g[:, :, : jg * P],
                    in_=feat_d[:, :, t0 * P : t0 * P + jg * P],
                )
                warm = t0 < 2  # the two 1-tile warm-up groups
                accb = accb_pool.tile([P, J, 2, MB, P], FP8)
                for j in range(jg):
                    nc.sync.dma_start(out=accb[:, j], in_=embB_d[t0 + j])
                if warm:
                    # Load the A-slots raw fp8 over HWDGE too: at startup the
                    # SWDGE cast stream is still ramping, and a raw load
                    # reaches the DVE ~5 us earlier; the 1x fp8 add cost on
                    # one tile is irrelevant here.
                    acca8 = acca8_pool.tile([P, 2, MA, P], FP8)
                    nc.sync.dma_start(out=acca8, in_=embA_d[t0])
                else:
                    acca = acca_pool.tile([P, J, 2, MA, P], F16)
                    for j in range(jg):
                        nc.gpsimd.dma_start(out=acca[:, j], in_=embA_d[t0 + j])

                # Binary add tree over the 16 neighbors, batched over the
                # group's jg tiles per instruction; every operand is packed
                # along the innermost node axis. L1 is split: raw-fp8 pairs
                # (1x, HWDGE -- lands first, so issued first) and f16 pairs
                # (2x) both land in s1's 8 slots.
                s1 = s1_pool.tile([P, J, 2, 8, P], F16)
                nc.vector.tensor_add(
                    out=s1[:, :jg, :, MA // 2 : 8],
                    in0=accb[:, :jg, :, 0 : MB // 2],
                    in1=accb[:, :jg, :, MB // 2 : MB],
                )
                if warm:
                    nc.vector.tensor_add(
                        out=s1[:, 0, :, 0 : MA // 2],
                        in0=acca8[:, :, 0 : MA // 2],
                        in1=acca8[:, :, MA // 2 : MA],
                    )
                else:
                    nc.vector.tensor_add(
                        out=s1[:, :jg, :, 0 : MA // 2],
                        in0=acca[:, :jg, :, 0 : MA // 2],
                        in1=acca[:, :jg, :, MA // 2 : MA],
                    )
                s2 = s2_pool.tile([P, J, 2, 4, P], F16)
                nc.vector.tensor_add(
                    out=s2[:, :jg], in0=s1[:, :jg, :, 0:4], in1=s1[:, :jg, :, 4:8]
                )
                s3 = s3_pool.tile([P, J, 2, 2, P], F16)
                nc.vector.tensor_add(
                    out=s3[:, :jg], in0=s2[:, :jg, :, 0:2], in1=s2[:, :jg, :, 2:4]
                )
                # (X @ W)^T = (s3[...,0] + s3[...,1] + feat) @ W, two
                # 128-row o-chunks; W chunk stationary, tree-sums/features
                # moving (jg*128 columns per matmul). The tree's last level
                # and the feat add ride the PSUM accumulation as extra matmul
                # passes -- the PE has headroom, the DVE is the wall.
                ps = ps_pool.tile([P, 2, J * P], F32)
                for oc in range(2):
                    for ci, (c, rep) in enumerate(
                        [
                            (0, s3[:, :jg, 0, 0]),
                            (1, s3[:, :jg, 1, 0]),
                            (0, s3[:, :jg, 0, 1]),
                            (1, s3[:, :jg, 1, 1]),
                            (0, feat_g[:, 0, : jg * P]),
                            (1, feat_g[:, 1, : jg * P]),
                        ]
                    ):
                        nc.tensor.matmul(
                            ps[:, oc, : jg * P],
                            w_sb[:, c, P * oc : P * (oc + 1)],
                            rep,
                            start=(ci == 0),
                            stop=(ci == 5),
                        )

                # relu(out^T + bias): bias is per-partition in this layout,
                # fused into the activation. f16 out.
                res = res_pool.tile([P, 2, J * P], F16)
                for oc in range(2):
                    nc.scalar.activation(
                        out=res[:, oc, : jg * P],
                        in_=ps[:, oc, : jg * P],
                        func=mybir.ActivationFunctionType.Relu,
                        bias=bias_sb[:, oc : oc + 1],
                    )
                nc.scalar.dma_start(
                    out=out_d[:, :, t0 * P : t0 * P + jg * P],
                    in_=res[:, :, : jg * P],
                )

    nc.finalize()
    return nc


def _make_in_maps(features, embedding_look_up, kernel, bias):
    """Marshal inputs: fold the neighbor-mean 1/16 into kernel, pre-scale
    features by 16, cast emb to fp8-e4m3 / the rest to f16, and pre-transpose
    emb ([t, p=d%128, c=d//128, m, n]) and features ([p=o%128, c, node]) so
    every device-side access is packed/contiguous."""
    import ml_dtypes

    feat = np.asarray(features, np.float32) * np.float32(MAX_NEIGH)
    emb8 = np.asarray(embedding_look_up, np.float32).astype(ml_dtypes.float8_e4m3)
    w_host = np.ascontiguousarray(
        (np.asarray(kernel, np.float32) / np.float32(MAX_NEIGH))
        .astype(np.float16)
        .reshape(2, P, D)
        .transpose(1, 0, 2)
    )
    bias_host = np.ascontiguousarray(
        np.asarray(bias, np.float32).reshape(2, P).T
    )

    nfull = (N_TILES - 1) * P  # 6144
    in_maps = []
    for cid in range(N_CORES):
        sl = slice(cid * PER_CORE, (cid + 1) * PER_CORE)
        e = emb8[sl].view(np.uint8)  # [6250, 16, 256]
        embT = np.empty((N_TILES, P, 2, MAX_NEIGH, P), dtype=np.uint8)
        blk = e[:nfull].reshape(N_TILES - 1, P, MAX_NEIGH, 2, P)  # [t, n, m, c, p]
        embT[: N_TILES - 1] = blk.transpose(0, 4, 3, 2, 1)
        embT[N_TILES - 1] = (
            e[TAIL0:PER_CORE].reshape(P, MAX_NEIGH, 2, P).transpose(3, 2, 1, 0)
        )
        embA = np.ascontiguousarray(embT[:, :, :, :MA])
        embB = np.ascontiguousarray(embT[:, :, :, MA:])

        ft = feat[sl]
        featT = np.empty((P, 2, N_PAD), dtype=np.float16)
        fblk = ft[:nfull].astype(np.float16).reshape(N_TILES - 1, P, 2, P)
        featT[:, :, :nfull] = fblk.transpose(3, 2, 0, 1).reshape(P, 2, nfull)
        featT[:, :, nfull:] = (
            ft[TAIL0:PER_CORE].astype(np.float16).reshape(P, 2, P).transpose(2, 1, 0)
        )

        in_maps.append(
            {
                "embA": embA.view(ml_dtypes.float8_e4m3),
                "embB": embB.view(ml_dtypes.float8_e4m3),
                "featT": featT,
                "w": w_host,
                "bias": bias_host,
            }
        )
    return in_maps


def _unpermute(res):
    """[128, 2, 6272] f16 (out^T, padded tiles) -> [6250, 256] f32."""
    nfull = (N_TILES - 1) * P
    tmp = res.transpose(1, 0, 2).reshape(D, N_PAD).astype(np.float32)  # [o, col]
    out = np.empty((PER_CORE, D), np.float32)
    out[:nfull] = tmp[:, :nfull].T
    out[TAIL0:PER_CORE] = tmp[:, nfull:].T
    return out


_NC_CACHE = None


def run(inputs: dict, trace: bool = False, fresh: bool = False):
    """Build, compile and run on 8 cores; returns (full_output, BassKernelResults)."""
    global _NC_CACHE
    in_maps = _make_in_maps(
        inputs["features"],
        inputs["embedding_look_up"],
        inputs["kernel"],
        inputs["bias"],
    )
    if fresh or _NC_CACHE is None:
        _NC_CACHE = build_nc()
    res = run_bass_kernel_spmd(
        _NC_CACHE, in_maps, core_ids=list(range(N_CORES)), trace=trace
    )
    out = np.concatenate([_unpermute(r["out"]) for r in res.results], axis=0)
    return out, res


def _spot_check(out, inputs) -> bool:
    """Cheap host-side check of 64 rows; catches (rare) silent device-side
    corruption so the caller can retry. fp8-emb pipeline error is ~1e-2."""
    idx = np.linspace(0, N_NODES - 1, 64).astype(np.int64)
    f = np.asarray(inputs["features"], np.float32)[idx]
    e = np.asarray(inputs["embedding_look_up"], np.float32)[idx]
    w = np.asarray(inputs["kernel"], np.float32)
    b = np.asarray(inputs["bias"], np.float32)
    exp = np.maximum((f + e.mean(axis=1)) @ w + b, 0.0)
    denom = max(np.abs(exp).max(), 1e-6)
    return np.abs(out[idx] - exp).max() / denom < 3e-2


def kernel(**inputs) -> np.ndarray:
    try:
        out, _ = run(inputs)
        if _spot_check(out, inputs):
            return out
    except Exception:
        # Transient NRT/device errors usually clear on a fresh attempt.
        pass
    out, _ = run(inputs, fresh=True)
    return out
